# revision 45
# baseline (speedup 1.0000x reference)
"""Trainium2 Bass kernel: causal multi-head attention with RoPE.

Reference computation (B=2, T=2048, C=2048, H=16, D=128, fp32):
    q/k/v = hs @ {q,k,v}_w^T ; RoPE(q), RoPE(k)
    out   = softmax(causal(q k^T / sqrt(D))) v @ o_w^T

Sharding: tensor-parallel over heads — each of the 8 cores owns 2 heads.
Each core computes its heads' projections + attention and a partial output
projection; the host sums the 8 partials (bf16 partials, fp32 sum).

Per-core device pipeline (bf16 data plane end to end: 1 cyc/row matmuls
with no minimum-N constraint, 2x DVE elementwise throughput via the
2x_1p perf mode, and half the SBUF traffic; PSUM accumulation stays
fp32):
  A) stream hs^T chunks; qT/kT in [d, t] layout (per-window tiles); v in
     [t, d] layout; RoPE (rotate_half as a +-1 permutation matmul +
     cos/sin elementwise, bf16 tables).  ~50 pre-warm matmuls on a
     zeroed tile open the PE HAM clock gate during the initial DMA wait,
     and the first window/batch uses per-chunk tiles with DMAs spread
     over 3 issue queues so the first matmul starts ~10us in.
  B) scores TRANSPOSED [tk, tq] with exact causal N-trimming: diagonal
     k-tile with offset `off` computes only columns tq >= 128*off; exp
     on ACT (bf16 out); triangle mask only on the boundary 128-wide
     block.  Softmax denominator: e-tiles pair- then QUAD-summed on DVE
     (bf16 2x), one ones-matmul per quad + per-diagonal-tile trimmed
     ones-matmuls, chained into one PSUM den row; DVE
     reciprocal_approx_fast (~5x faster than InstReciprocal, 18-bit);
     gpsimd partition-broadcast; normalize fused into the PSUM->SBUF
     attnT copy and deferred one attend-step.  Windows processed
     [1,2,3,0]; the final window of the last batch is attended in four
     COLUMN QUARTERS so each quarter's output projection + DMA overlap
     the next quarter's attention (the end-of-kernel drain is one m-row).
  C) output projection interleaved between attention windows at
     half-window granularity; per m-row the four [128,512] psum tiles
     are copied (ACT/DVE alternating) into one [128, 2048] bf16 staging
     tile and shipped with a SINGLE output DMA (4KB contiguous per
     partition); the last window's DMAs all go via the sync queue so
     gpsimd's end-of-program DRAINs have nothing outstanding.  Host sums
     the 8 bf16 partials in fp32.
"""

import math
import sys

if "/opt/trn_rl_repo" not in sys.path:
    sys.path.insert(0, "/opt/trn_rl_repo")

import numpy as np

import concourse.bass as bass
import concourse.mybir as mybir
import concourse.tile as tile
from concourse import bacc, bass_utils

F32 = mybir.dt.float32
F32R = mybir.dt.float32r
BF16 = mybir.dt.bfloat16
AF = mybir.ActivationFunctionType
MULT = mybir.AluOpType.mult
ADD = mybir.AluOpType.add
B = 2
C = 2048
H = 16
D = 128
N_CORES = 8
HPC = H // N_CORES  # heads per core
DPC = HPC * D  # channels per core (256)
ROPE_BASE = 10000.0
P = 128  # partitions
TQW = 512  # tq window (matmul free dim)
TCH = 256  # hs^T chunk width in t


def _build_nc(T: int = 2048):
    """Build the per-core Bass program (SPMD: same program, per-core data)."""
    KT = C // P  # 16 k-tiles over the contraction dim c
    n_w = T // TQW  # tq windows per (b, h)
    spw = TQW // P  # 128-row subtiles per window (4)
    scale = 1.0 / math.sqrt(D)

    nc = bacc.Bacc(trn_type="TRN2", target_bir_lowering=False, debug=False)

    hst = nc.dram_tensor("hst", [B, P, T // TQW, KT // 4, 4, TQW], BF16, kind="ExternalInput").ap()
    wq = nc.dram_tensor("wq_t", [P, KT, DPC], BF16, kind="ExternalInput").ap()
    wk = nc.dram_tensor("wk_t", [P, KT, DPC], BF16, kind="ExternalInput").ap()
    wv = nc.dram_tensor("wv_t", [P, KT, DPC], BF16, kind="ExternalInput").ap()
    ow = nc.dram_tensor("ow_t", [P, HPC, C], BF16, kind="ExternalInput").ap()
    cos_d = nc.dram_tensor("cos_t", [D, T], BF16, kind="ExternalInput").ap()
    sin_d = nc.dram_tensor("sin_t", [D, T], BF16, kind="ExternalInput").ap()
    rp_d = nc.dram_tensor("rperm", [D, D], BF16, kind="ExternalInput").ap()
    ones_d = nc.dram_tensor("ones", [P, 1], BF16, kind="ExternalInput").ap()
    msk_d = nc.dram_tensor("masks", [P, P], BF16, kind="ExternalInput").ap()
    out_d = nc.dram_tensor("out_p", [B, T // P, P, C // TQW, TQW], BF16, kind="ExternalOutput").ap()

    with tile.TileContext(nc) as tc:
        with (
            tc.tile_pool(name="consts", bufs=1) as consts,
            tc.tile_pool(name="hst", bufs=12) as hstp,
            tc.tile_pool(name="qkv", bufs=1) as qkvp,
            tc.tile_pool(name="exp", bufs=6) as expp,
            tc.tile_pool(name="esum", bufs=4) as esump,
            tc.tile_pool(name="bc", bufs=3) as bcp,
            tc.tile_pool(name="small", bufs=2) as smallp,
            tc.tile_pool(name="outp", bufs=4) as outp,
            tc.tile_pool(name="psA", bufs=4, space="PSUM") as psA,
            tc.tile_pool(name="psB", bufs=4, space="PSUM") as psB,
        ):
            # ---- resident constants -------------------------------------
            # First 4 contraction chunks of each weight are separate tiles
            # (fine-grained arrival for the kernel head); the remaining 12
            # keep quarter granularity.
            wq_c = [consts.tile([P, DPC], BF16, tag=f"wqc{k}", name=f"wqc{k}") for k in range(4)]
            wk_c = [consts.tile([P, DPC], BF16, tag=f"wkc{k}", name=f"wkc{k}") for k in range(4)]
            wv_c = [consts.tile([P, DPC], BF16, tag=f"wvc{k}", name=f"wvc{k}") for k in range(4)]
            wq_q = [consts.tile([P, 4, DPC], BF16, tag=f"wqq{i}", name=f"wqq{i}") for i in range(1, 4)]
            wk_q = [consts.tile([P, 4, DPC], BF16, tag=f"wkq{i}", name=f"wkq{i}") for i in range(1, 4)]
            wv_q = [consts.tile([P, 4, DPC], BF16, tag=f"wvq{i}", name=f"wvq{i}") for i in range(1, 4)]
            ow_sb = consts.tile([P, HPC, C], BF16, tag="ow")
            cos_sb = consts.tile([D, T], BF16, tag="cos")
            sin_sb = consts.tile([D, T], BF16, tag="sin")
            msk_sb = consts.tile([P, P], BF16, tag="msk")
            ones_sb = consts.tile([P, 1], BF16, tag="ones")
            rp_sb = consts.tile([D, D], BF16, tag="rp")

            def wslc(w_c, w_q, k, h=None):
                if k < 4:
                    t = w_c[k]
                    return t[:, bass.ts(h, D)] if h is not None else t[:]
                t = w_q[k // 4 - 1]
                return (
                    t[:, k % 4, bass.ts(h, D)] if h is not None else t[:, k % 4, :]
                )

            # Critical-path-first DMA order, spread over FOUR issue queues
            # (scalar/gpsimd/vector/sync) in consumption order so the first
            # matmuls (k=0..3 of window 0) can start early and the k=4..15
            # weight quarters land before the k-loop reaches them.  The
            # first window's hs^T quarters are DMA'd per chunk (4 smaller
            # transfers into slices of each quarter tile).
            pre_tiles = [
                hstp.tile([P, 4, TQW], BF16, tag="hst", name="ht_pre")
                for _ in range(4)
            ]
            # PE pre-warm: ~50 matmuls on a zeroed tile keep the PE busy
            # during the initial DMA wait so the HAM clock-gate reaches
            # K=8/8 before real work arrives (saves the 2x-slow cold
            # stretch at the head of phase A).
            warm = consts.tile([P, 256], BF16, tag="warm")
            nc.vector.memset(warm[:], 0)
            for _wi in range(50):
                wps = psB.tile([P, 256], F32, tag="psB", name="warm")
                nc.tensor.matmul(
                    wps[:], warm[:, 0:P], warm[:], start=True, stop=True
                )
            nc.scalar.dma_start(wq_c[0][:], wq[:, 0, :])
            nc.gpsimd.dma_start(wk_c[0][:], wk[:, 0, :])
            nc.sync.dma_start(pre_tiles[0][:, 0, :], hst[0, :, 0, 0, 0, :])
            nc.scalar.dma_start(wq_c[1][:], wq[:, 1, :])
            nc.gpsimd.dma_start(wk_c[1][:], wk[:, 1, :])
            nc.sync.dma_start(pre_tiles[0][:, 1, :], hst[0, :, 0, 0, 1, :])
            nc.scalar.dma_start(wq_c[2][:], wq[:, 2, :])
            nc.scalar.dma_start(wk_c[2][:], wk[:, 2, :])
            nc.sync.dma_start(pre_tiles[0][:, 2, :], hst[0, :, 0, 0, 2, :])
            nc.scalar.dma_start(wq_c[3][:], wq[:, 3, :])
            nc.scalar.dma_start(wk_c[3][:], wk[:, 3, :])
            nc.sync.dma_start(pre_tiles[0][:, 3, :], hst[0, :, 0, 0, 3, :])
            nc.gpsimd.dma_start(wk_q[0][:], wk[:, bass.ts(1, 4), :])
            nc.scalar.dma_start(wq_q[0][:], wq[:, bass.ts(1, 4), :])
            for kk in range(4):
                nc.sync.dma_start(pre_tiles[1][:, kk, :], hst[0, :, 0, 1, kk, :])
            nc.gpsimd.dma_start(wk_q[1][:], wk[:, bass.ts(2, 4), :])
            nc.scalar.dma_start(wq_q[1][:], wq[:, bass.ts(2, 4), :])
            nc.sync.dma_start(pre_tiles[2][:], hst[0, :, 0, 2, :, :])
            nc.gpsimd.dma_start(wk_q[2][:], wk[:, bass.ts(3, 4), :])
            nc.scalar.dma_start(wq_q[2][:], wq[:, bass.ts(3, 4), :])
            nc.sync.dma_start(pre_tiles[3][:], hst[0, :, 0, 3, :, :])
            nc.scalar.dma_start(rp_sb[:], rp_d)
            for k in range(4):
                nc.sync.dma_start(wv_c[k][:], wv[:, k, :])
            for i in range(3):
                nc.sync.dma_start(wv_q[i][:], wv[:, bass.ts(i + 1, 4), :])
            nc.scalar.dma_start(cos_sb[:], cos_d)
            nc.scalar.dma_start(sin_sb[:], sin_d)
            nc.scalar.dma_start(msk_sb[:], msk_d)
            nc.scalar.dma_start(ones_sb[:], ones_d)
            late_dmas_done = []

            for b in range(B):
                # Per-window q/k tiles: fine-grained deps (a window's
                # consumers only wait on that window's producers).
                q_t = [
                    [qkvp.tile([P, TQW], BF16, tag=f"q{h}w{w}", name=f"q{h}w{w}") for w in range(n_w)]
                    for h in range(HPC)
                ]
                k_t = [
                    [qkvp.tile([P, TQW], BF16, tag=f"k{h}w{w}", name=f"k{h}w{w}") for w in range(n_w)]
                    for h in range(HPC)
                ]
                v_sb = qkvp.tile([P, T // P, DPC], BF16, tag="v")

                # ---- phase A: projections + RoPE ------------------------
                def rope(w, b=b, ps=None):
                    sl = bass.ts(w, TQW)
                    for h in range(HPC):
                        for x_t in (q_t, k_t):
                            x = x_t[h][w]
                            pool = ps or psB
                            rh = pool.tile([P, TQW], F32, tag=pool.name, name="rh")
                            nc.tensor.matmul(
                                rh[:], rp_sb[:], x[:], start=True, stop=True
                            )
                            # t1 = x*cos (all-bf16: 2x DVE); rh2 = rh*sin
                            # (psum f32 x bf16 -> bf16); x = t1+rh2 (2x DVE)
                            t1 = smallp.tile([P, TQW], BF16, tag="t1")
                            nc.vector.tensor_tensor(
                                t1[:], x[:], cos_sb[:, sl], op=MULT
                            )
                            rh2 = smallp.tile([P, TQW], BF16, tag="t2")
                            nc.vector.tensor_tensor(rh2[:], rh[:], sin_sb[:, sl], op=MULT)
                            nc.vector.tensor_tensor(x[:], t1[:], rh2[:], op=ADD)

                ctx_a = nc.named_scope(f"A{b}"); ctx_a.__enter__()
                for w in range(n_w):
                    if b == 0 and w == 0:
                        hts = pre_tiles
                    else:
                        hts = []
                        for qi in range(4):
                            ht = hstp.tile([P, 4, TQW], BF16, tag="hst", name="ht")
                            nc.sync.dma_start(ht[:], hst[b, :, w, qi, :, :])
                            hts.append(ht)
                    hsl = [hts[k // 4][:, k % 4, :] for k in range(KT)]
                    hsl_sub = lambda k, sub, hts=hts: hts[k // 4][:, k % 4, bass.ts(sub, P)]
                    pq = [psA.tile([P, TQW], F32, tag="psA", name="pq") for _ in range(HPC)]
                    pk = [psA.tile([P, TQW], F32, tag="psA", name="pk") for _ in range(HPC)]
                    for k in range(KT):
                        for h in range(HPC):
                            for pt, w_cq in ((pq[h], (wq_c, wq_q)), (pk[h], (wk_c, wk_q))):
                                nc.tensor.matmul(
                                    pt[:],
                                    wslc(w_cq[0], w_cq[1], k, h),
                                    hsl[k],
                                    start=(k == 0),
                                    stop=(k == KT - 1),
                                )
                    # Rank the psum->sbuf copies later so attention's first
                    # exps win the ACT queue at the phase A->B transition
                    # (deps still force early-window copies on time).
                    with tc.high_priority(-2000):
                        for h in range(HPC):
                            nc.scalar.activation(q_t[h][w][:], pq[h][:], AF.Copy)
                            nc.scalar.activation(k_t[h][w][:], pk[h][:], AF.Copy)
                    pv4 = [
                        psB.tile([P, DPC], F32, tag="psB", name="pv4")
                        for _ in range(spw)
                    ]

                    def v_mms(k0, k1, hsl_sub=hsl_sub, pv4=pv4):
                        for k in range(k0, k1):
                            for sub in range(spw):
                                nc.tensor.matmul(
                                    pv4[sub][:],
                                    hsl_sub(k, sub),
                                    wslc(wv_c, wv_q, k),
                                    start=(k == 0),
                                    stop=(k == KT - 1),
                                )

                    # For the LAST window the second half of the v matmuls
                    # is emitted after rope(), so the PE has work while the
                    # final rope DVE chain drains (otherwise that chain
                    # exposes a PE gap at the attention-phase start).  rope
                    # then takes its rh PSUM tiles from psA -- pv4 holds all
                    # four psB ring slots across rope, and an rh allocation
                    # from psB would deadlock: its WAR release (the v
                    # copies) sits behind rh itself on the FIFO PE queue.
                    last = w == n_w - 1
                    v_mms(0, KT // 2 if last else KT)
                    rope(w, ps=psA if last else None)
                    if last:
                        v_mms(KT // 2, KT)
                    with tc.high_priority(-2000):
                        for sub in range(spw):
                            # GPSIMD cannot read PSUM; DVE takes these
                            nc.vector.tensor_copy(
                                v_sb[:, w * spw + sub, :], pv4[sub][:]
                            )
                ctx_a.__exit__(None, None, None)

                # ---- phase B: attention -------------------------------
                # Diagonal k-tile column trim: tile with offset `off`
                # (0..3) only needs columns tq >= 128*off (bf16 matmuls
                # have no minimum-N penalty, so the trim is exact).  The
                # boundary 128-wide block at [c0:c0+128] gets the
                # triangle mask; columns above it are fully valid.
                def attend_win(h, w, c_lo=0, c_hi=TQW):
                    # Attend query columns [c_lo, c_hi) of window w.  The
                    # final window of the last batch is processed in two
                    # column halves so its output projection (and output
                    # DMA) overlaps the second half's attention.
                    ntk = w * spw + min(spw, c_hi // P)

                    def qk_exp(i, h=h, w=w):
                        off = i - w * spw
                        c0 = max(c_lo, 0 if off <= 0 else P * off)
                        st = psB.tile([P, TQW], F32, tag="psB")
                        nc.tensor.matmul(
                            st[:, c0:c_hi],
                            k_t[h][i // spw][:, bass.ts(i % spw, P)],
                            q_t[h][w][:, c0:c_hi],
                            start=True,
                            stop=True,
                        )
                        e = expp.tile([P, TQW], BF16, tag="exp")
                        nc.scalar.activation(
                            e[:, c0:c_hi], st[:, c0:c_hi], AF.Exp, scale=scale
                        )
                        if off >= 0 and P * off >= c_lo:
                            o0 = P * off
                            nc.vector.tensor_tensor(
                                e[:, o0:o0 + P], e[:, o0:o0 + P],
                                msk_sb[:], op=MULT,
                            )
                        return e, c0

                    fifo = [qk_exp(j) for j in range(min(3, ntk))]
                    pv = psA.tile([P, TQW], F32, tag="psA")
                    den = psA.tile([P, TQW], F32, tag="psA")
                    # leading tiles whose mask block is below c_lo (no mask,
                    # full [c_lo, c_hi) width) -- eligible for pair/quad sums
                    nfull = w * spw + min(spw, max(0, c_lo // P))
                    # Denominator: DVE pair- then quad-sums of full-width
                    # e-tiles feed one ones-matmul per QUAD; the diagonal
                    # tiles get individual N-trimmed ones-matmuls.  Jobs are
                    # emitted one behind the PV stream so nothing stalls.
                    # Natural order puts a [c_lo, c_hi)-covering operand
                    # first, as required for start=True (has_written).
                    pair_pend = None
                    quad_pend = None
                    jobs = []  # (ap, c0) pending den matmuls
                    quads, rem = divmod(nfull, 4)
                    n_jobs_total = quads + (rem + 1) // 2 + (ntk - nfull)
                    emitted = [0]

                    def den_mm():
                        ap, c0 = jobs.pop(0)
                        nc.tensor.matmul(
                            den[:1, c0:c_hi],
                            ones_sb[:],
                            ap,
                            start=(emitted[0] == 0),
                            stop=(emitted[0] == n_jobs_total - 1),
                        )
                        emitted[0] += 1

                    for i in range(ntk):
                        if i + 3 < ntk:
                            fifo.append(qk_exp(i + 3))
                        e, c0 = fifo.pop(0)
                        nc.tensor.matmul(
                            pv[:, c0:c_hi],
                            v_sb[:, i, bass.ts(h, D)],
                            e[:, c0:c_hi],
                            start=(i == 0),
                            stop=(i == ntk - 1),
                        )
                        if i < nfull:
                            if pair_pend is None:
                                pair_pend = e
                            else:
                                pr = esump.tile(
                                    [P, TQW], BF16, tag="pair", name="pair"
                                )
                                nc.vector.tensor_tensor(
                                    pr[:, c_lo:c_hi],
                                    pair_pend[:, c_lo:c_hi],
                                    e[:, c_lo:c_hi],
                                    op=ADD,
                                )
                                pair_pend = None
                                if quad_pend is None:
                                    quad_pend = pr
                                else:
                                    qd = esump.tile(
                                        [P, TQW], BF16, tag="pair", name="quad"
                                    )
                                    nc.vector.tensor_tensor(
                                        qd[:, c_lo:c_hi],
                                        quad_pend[:, c_lo:c_hi],
                                        pr[:, c_lo:c_hi],
                                        op=ADD,
                                    )
                                    quad_pend = None
                                    jobs.append((qd[:, c_lo:c_hi], c_lo))
                            if i == nfull - 1:
                                # flush leftovers (quad first: it covers
                                # [c_lo, c_hi) when it is the first job)
                                if quad_pend is not None:
                                    jobs.append(
                                        (quad_pend[:, c_lo:c_hi], c_lo)
                                    )
                                    quad_pend = None
                                if pair_pend is not None:
                                    jobs.append(
                                        (pair_pend[:, c_lo:c_hi], c_lo)
                                    )
                                    pair_pend = None
                        else:
                            jobs.append((e[:, c0:c_hi], c0))
                        # drain den jobs, keeping two in flight for slack
                        while len(jobs) > 2:
                            den_mm()
                    while jobs:
                        den_mm()

                    # reciprocal immediately (frees the den PSUM slot);
                    # broadcast + normalize are deferred one attend-step by
                    # the caller to avoid engine convoys.  approx_fast is
                    # ~5x faster than InstReciprocal and accurate to ~18
                    # bits -- far beyond what softmax normalization needs.
                    bc = bcp.tile([P, TQW], F32, tag="bc", name="bc")
                    nc.vector.reciprocal_approx_fast(
                        bc[:1, c_lo:c_hi], den[:1, c_lo:c_hi]
                    )

                    def finalize(h=h, w=w, pv=pv, bc=bc, c_lo=c_lo, c_hi=c_hi):
                        nc.gpsimd.partition_broadcast(
                            bc[:, c_lo:c_hi], bc[:1, c_lo:c_hi]
                        )
                        nc.vector.tensor_tensor(
                            q_t[h][w][:, c_lo:c_hi],
                            pv[:, c_lo:c_hi],
                            bc[:, c_lo:c_hi],
                            op=MULT,
                        )
                    return finalize

                def phase_c_win(w, half=None, sync_dma=False, msel=None):
                    ms = range(w * spw, (w + 1) * spw)
                    if msel is not None:
                        ms = [w * spw + i for i in msel]
                    elif half is not None:
                        ms = ms[: len(ms) // 2] if half == 0 else ms[len(ms) // 2 :]
                    for m in ms:
                        # One [P, 4, TQW] staging tile and ONE output DMA per
                        # m-row (4KB contiguous per partition): 16 output
                        # DMAs kernel-wide instead of 64, which unclogs the
                        # issue queues at the kernel tail.
                        o_t = outp.tile([P, C // TQW, TQW], BF16, tag="o")
                        for n in range(C // TQW):
                            pool = psA if n % 2 == 0 else psB
                            po = pool.tile([P, TQW], F32, tag=pool.name, name="po")
                            for h in range(HPC):
                                nc.tensor.matmul(
                                    po[:],
                                    q_t[h][m // spw][:, bass.ts(m % spw, P)],
                                    ow_sb[:, h, bass.ts(n, TQW)],
                                    start=(h == 0),
                                    stop=(h == HPC - 1),
                                )
                            if n % 2 == 0:
                                with tc.high_priority(-1500):
                                    nc.scalar.activation(o_t[:, n, :], po[:], AF.Copy)
                            else:
                                nc.vector.tensor_copy(o_t[:, n, :], po[:])
                        # the very last window's outputs all go via the sync
                        # queue so the gpsimd queue's end-of-program DRAINs
                        # have nothing left outstanding to wait on
                        eng = nc.sync if (sync_dma or m % 2 == 0) else nc.gpsimd
                        eng.dma_start(out_d[b, m], o_t[:])

                # ---- attention + output projection, software-pipelined:
                # phase C of the previously processed window runs between
                # attention windows so output DMA overlaps compute.  The
                # cheapest window (0) goes last to minimize the tail.
                if not late_dmas_done:
                    nc.sync.dma_start(ow_sb[:], ow)
                    late_dmas_done.append(True)
                with nc.named_scope(f"BC{b}"):
                    wins = [1, 2, 3, 0] if n_w == 4 else list(range(1, n_w)) + [0]
                    pending = []
                    for idx, w in enumerate(wins[:-1]):
                        pending.append(attend_win(0, w))
                        if len(pending) > 1:
                            pending.pop(0)()
                        if idx > 0:
                            phase_c_win(wins[idx - 1], half=0)
                        pending.append(attend_win(1, w))
                        if len(pending) > 1:
                            pending.pop(0)()
                        if idx > 0:
                            phase_c_win(wins[idx - 1], half=1)
                    wl, prev = wins[-1], wins[-2]
                    if b < B - 1:
                        pending.append(attend_win(0, wl))
                        pending.pop(0)()
                        phase_c_win(prev, half=0)
                        pending.append(attend_win(1, wl))
                        pending.pop(0)()
                        phase_c_win(prev, half=1)
                        pending.pop(0)()
                        phase_c_win(wl)
                    else:
                        # Last batch: attend the final window in column
                        # QUARTERS so each quarter's output projection +
                        # DMA overlaps the next quarter's attention -- the
                        # end-of-kernel drain is a single m-row.
                        QW = TQW // 4
                        qr = [(j * QW, (j + 1) * QW) for j in range(4)]
                        pending.append(attend_win(0, wl, *qr[0]))
                        pending.pop(0)()
                        phase_c_win(prev, msel=[0])
                        pending.append(attend_win(1, wl, *qr[0]))
                        pending.pop(0)()
                        phase_c_win(prev, msel=[1])
                        pending.append(attend_win(0, wl, *qr[1]))
                        pending.pop(0)()
                        phase_c_win(prev, msel=[2])
                        pending.append(attend_win(1, wl, *qr[1]))
                        pending.pop(0)()
                        phase_c_win(prev, msel=[3])
                        pending.append(attend_win(0, wl, *qr[2]))
                        pending.pop(0)()
                        phase_c_win(wl, msel=[0], sync_dma=True)
                        pending.append(attend_win(1, wl, *qr[2]))
                        pending.pop(0)()
                        phase_c_win(wl, msel=[1], sync_dma=True)
                        pending.append(attend_win(0, wl, *qr[3]))
                        pending.pop(0)()
                        phase_c_win(wl, msel=[2], sync_dma=True)
                        pending.append(attend_win(1, wl, *qr[3]))
                        pending.pop(0)()
                        pending.pop(0)()
                        phase_c_win(wl, msel=[3], sync_dma=True)

    nc.compile()
    return nc


def _host_prep(hidden_states, q_w, k_w, v_w, o_w):
    """Build the 8 per-core input maps (and shared constant tensors)."""
    T = hidden_states.shape[1]
    f32 = np.float32

    n_w = T // TQW
    KT = C // P
    # [B, T, C] -> hs^T blocked per (partition, window, k-quarter):
    # [B, P, n_w, KT//4, 4, TQW]
    hstT = hidden_states.transpose(0, 2, 1)  # [B, C, T]
    hst = np.ascontiguousarray(
        hstT.reshape(B, KT // 4, 4, P, n_w, TQW).transpose(0, 3, 4, 1, 2, 5)
    ).astype(f32, copy=False)

    def wblk(w_slice):
        # [DPC, C] row-slice -> w^T blocked [P, KT, DPC]
        return np.ascontiguousarray(
            w_slice.T.reshape(KT, P, DPC).transpose(1, 0, 2)
        ).astype(f32, copy=False)

    inv_freq = 1.0 / (ROPE_BASE ** (np.arange(0, D, 2, dtype=np.float64) / D))
    t_ar = np.arange(T, dtype=np.float64)
    freqs = t_ar[:, None] * inv_freq[None, :]  # [T, D/2]
    cos_td = np.concatenate([np.cos(freqs), np.cos(freqs)], axis=-1)  # [T, D]
    sin_td = np.concatenate([np.sin(freqs), np.sin(freqs)], axis=-1)
    cos_t = np.ascontiguousarray(cos_td.T).astype(f32)  # [D, T]
    sin_t = np.ascontiguousarray(sin_td.T).astype(f32)

    # rotate_half as a matmul: rh = R @ x ; rperm = R^T (lhsT operand).
    rperm = np.zeros((D, D), dtype=f32)
    half = D // 2
    for j in range(half):
        rperm[2 * j + 1, j] = -1.0
    for j in range(half, D):
        rperm[2 * (j - half), j] = 1.0

    ones = np.ones((P, 1), dtype=f32)

    # masks: boundary-block triangle (col >= row).
    masks = (np.arange(P)[None, :] >= np.arange(P)[:, None]).astype(f32)

    import ml_dtypes
    bf16 = ml_dtypes.bfloat16
    hst = hst.astype(bf16)

    in_maps = []
    for c in range(N_CORES):
        rs, re = c * DPC, (c + 1) * DPC
        in_maps.append(
            {
                "hst": hst,
                "wq_t": wblk(q_w[rs:re, :]).astype(bf16),
                "wk_t": wblk(k_w[rs:re, :]).astype(bf16),
                "wv_t": wblk(v_w[rs:re, :]).astype(bf16),
                "ow_t": np.ascontiguousarray(
                    o_w[:, rs:re].T.reshape(HPC, P, C).transpose(1, 0, 2)
                ).astype(bf16),
                "cos_t": cos_t.astype(bf16),
                "sin_t": sin_t.astype(bf16),
                "rperm": rperm.astype(bf16),
                "ones": ones.astype(bf16),
                "masks": masks.astype(bf16),
            }
        )
    return in_maps


_NC_CACHE = {}


def _get_nc(T):
    if T not in _NC_CACHE:
        _NC_CACHE[T] = _build_nc(T)
    return _NC_CACHE[T]


def kernel(hidden_states, q_w, k_w, v_w, o_w, **run_kwargs):
    hidden_states = np.asarray(hidden_states, dtype=np.float32)
    q_w = np.asarray(q_w, dtype=np.float32)
    k_w = np.asarray(k_w, dtype=np.float32)
    v_w = np.asarray(v_w, dtype=np.float32)
    o_w = np.asarray(o_w, dtype=np.float32)
    T = hidden_states.shape[1]
    nc = _get_nc(T)
    in_maps = _host_prep(hidden_states, q_w, k_w, v_w, o_w)
    res = bass_utils.run_bass_kernel_spmd(
        nc, in_maps, core_ids=list(range(N_CORES)), **run_kwargs
    )
    out = np.zeros((B, T // P, P, C // TQW, TQW), dtype=np.float32)
    for r in res.results:
        out += np.asarray(r["out_p"]).astype(np.float32)
    kernel.last_results = res
    return out.reshape(B, T, C).astype(np.float32)

